# revision 9
# baseline (speedup 1.0000x reference)
"""Trainium2 Bass kernel for nn_DeltaModel (histogram_binning) — fused single-launch.

Reference semantics (delta == 0, the shipped configuration):
  med[t,ch]   = lower median over N of logits[t,:,ch]          (rows 0-4 used)
  std[n,ch]   = unbiased std over the 10 rows
  std_med[ch] = lower median over N of std[:,ch]
  T[t,ch]     = med[t,ch] + 1.96*std_med[ch]
  mode[n,ch]  = (#{t<5: logits[t,n,ch] >= T[t,ch]} >= 3)
  c           = broadcast(mode) over dim 0
  out[t,:,ch] = xs[t,ch] - logsumexp(xs[t,others(ch)])  (constant over N)

Device work is ONE SPMD launch over 8 NeuronCores. Each core streams its
column shard once and produces:
  q[n,ch]  = sumsq - 0.1*sum^2 over the 10 rows  (== 9*var, monotone in std)
  pk[n,ch] = cnt_lo + 8*cnt_hi, where cnt_lo/hi count rows t<5 with
             x >= T_est[t,ch] -/+ DELTA_BRK  (a bracket around the true T)
T_est comes from a host-side subsample; the bracket makes the device counts
decide mode EXACTLY for every column whose counts agree on the >=3 boundary
(all but a few hundred of the 4M). Host does the exact order statistics on
small arrays: med via np.partition on the raw logits (bit-exact vs the
reference sort, overlapped with the launch), qmed via np.partition on the
gathered q. Straddling columns are re-resolved exactly from the raw logits.
Outputs are assembled as broadcast views (out is constant along N at
delta == 0; c broadcasts mode over dim 0).
"""

import os
import threading

import numpy as np

LAST_RUN_TIMES = []  # wall seconds of each device launch (incl. first-call compile)

N = 1_000_000
NCORES = 8
SHARD = N // NCORES            # 125000
PADW_PP = 490                  # per-partition padded columns (5 x 98), per half
SHARD_PAD = 128 * PADW_PP      # 62720
SHARD_H = SHARD // 2           # 62500 columns per core per half-launch
NROWS = 10
NCH = 4
FACTOR = np.float32(1.96)
DELTA_BRK = np.float32(0.03)   # half-width of the threshold bracket
NITERS = 5

_JAX_CACHE_DIR = "/root/.jax_bass_cache"


def _enable_jax_cache():
    try:
        import jax
        os.makedirs(_JAX_CACHE_DIR, exist_ok=True)
        jax.config.update("jax_compilation_cache_dir", _JAX_CACHE_DIR)
        jax.config.update("jax_persistent_cache_min_entry_size_bytes", 0)
        jax.config.update("jax_persistent_cache_min_compile_time_secs", 0.0)
    except Exception:
        pass


def _apply_tile_patch():
    """This walrus build rejects >2 sync waits on the SP Drain emitted at
    TileContext exit ("Too many sync wait commands"); keep one wait on the
    drain and move the rest onto dedicated SP nops before the barrier."""
    import concourse.tile as tile_mod
    from concourse import mybir
    from concourse.vector_clock import ScopedClock

    if getattr(tile_mod.TileContext, "_ant_drain_patched", False):
        return

    def _patched(self, tick_clock, wait_clock):
        nc = self.nc
        drain_inst = nc.sync.drain()
        wait_clock.add_sem_waits(
            drain_inst.ins, ScopedClock({None: tick_clock.global_clock})
        )
        si = drain_inst.ins.sync_info
        if si is not None and si.on_wait is not None and len(si.on_wait) > 1:
            waits = list(si.on_wait)
            drain_inst.ins.sync_info = mybir.SyncInfo(
                on_wait=waits[:1], on_update=list(si.on_update or [])
            )
            for w in waits[1:]:
                nop = nc.sync.nop()
                nop.ins.sync_info = mybir.SyncInfo(on_wait=[w], on_update=[])
        nc.all_engine_barrier()
        assert self.sems is not None
        popped = nc._tile_sem_poison_stack.pop()
        assert popped is self._sem_poison
        nc.clear_and_free_semaphores(list(self.sems.allocated().values()))
        nc.all_engine_barrier()

    tile_mod.TileContext._drain_and_barrier = _patched
    tile_mod.TileContext._ant_drain_patched = True


def _split_sync_waits(nc, maxw=1):
    """This walrus build caps per-instruction sync waits; move excess waits
    onto same-engine NoOps inserted right before the offending instruction."""
    from concourse import mybir

    for f in nc.m.functions:
        for b in f.blocks:
            new_list = []
            changed = False
            for ins in b.instructions:
                si = getattr(ins, "sync_info", None)
                if si is not None and si.on_wait and len(si.on_wait) > maxw:
                    waits = list(si.on_wait)
                    extra, keep = waits[:-maxw], waits[-maxw:]
                    for i in range(0, len(extra), maxw):
                        nop = mybir.InstNoOp(
                            name=f"{ins.name}-wsplit{i}", ins=[], outs=[]
                        )
                        nop.engine = ins.engine
                        nop.sync_info = mybir.SyncInfo(
                            on_wait=extra[i:i + maxw], on_update=[]
                        )
                        new_list.append(nop)
                        changed = True
                    ins.sync_info = mybir.SyncInfo(
                        on_wait=keep, on_update=list(si.on_update or [])
                    )
                new_list.append(ins)
            if changed:
                b.instructions = new_list


def _build_warmup():
    """Trivial program: touches all 8 cores so the first real launch finds a
    warm execution path."""
    import concourse.bass as bass
    import concourse.tile as tile
    from concourse import mybir

    _apply_tile_patch()
    nc = bass.Bass("TRN2", target_bir_lowering=False, debug=False, num_devices=1)
    inp = nc.dram_tensor("inp", [128, 128], mybir.dt.float32,
                         kind="ExternalInput").ap()
    outp = nc.dram_tensor("outp", [128, 128], mybir.dt.float32,
                          kind="ExternalOutput").ap()
    with tile.TileContext(nc) as tc:
        with tc.tile_pool(name="p", bufs=1) as pool:
            t = pool.tile([128, 128], mybir.dt.float32)
            nc.sync.dma_start(out=t, in_=inp)
            nc.vector.tensor_scalar(out=t, in0=t, scalar1=1.0, scalar2=None,
                                    op0=mybir.AluOpType.mult)
            nc.sync.dma_start(out=outp, in_=t)
    _split_sync_waits(nc)
    return nc


_warmup_thread = None


def _warmup():
    try:
        from concourse.bass_utils import run_bass_kernel_spmd
        nc = _WARMUP_NC
        a = np.ones((128, 128), np.float32)
        run_bass_kernel_spmd(nc, [{"inp": a}] * NCORES,
                             core_ids=list(range(NCORES)))
    except Exception:
        pass


def _start_warmup():
    global _warmup_thread
    if _warmup_thread is None:
        _warmup_thread = threading.Thread(target=_warmup, daemon=True)
        _warmup_thread.start()


_enable_jax_cache()
try:
    # Build sequentially at import (bass builder state stays deterministic),
    # then run it on a background thread so device/session init overlaps the
    # caller's input loading.
    _WARMUP_NC = _build_warmup()
    _start_warmup()
except Exception:
    _WARMUP_NC = None


def build_fused(niters=NITERS, padw_pp=PADW_PP, split_waits=True):
    """One pass over the shard: q = ssq - 0.1*sum^2 (all 10 rows, PE-reduced)
    and packed bracket counts over rows 0-4 (DVE)."""
    import concourse.bass as bass
    import concourse.tile as tile
    from concourse import mybir

    _apply_tile_patch()
    chunk = padw_pp // niters
    free = chunk * NCH
    qw = padw_pp * NCH
    nc = bass.Bass("TRN2", target_bir_lowering=False, debug=False, num_devices=1)
    shard = nc.dram_tensor("shardpad", [NROWS, 128 * padw_pp, NCH], mybir.dt.float32,
                           kind="ExternalInput").ap()
    th = nc.dram_tensor("th", [2, 5, NCH], mybir.dt.float32,
                        kind="ExternalInput").ap()
    identd = nc.dram_tensor("ident", [128, 128], mybir.dt.float32,
                            kind="ExternalInput").ap()
    qvar = nc.dram_tensor("qvar", [128, qw], mybir.dt.float32,
                          kind="ExternalOutput").ap()
    cnto = nc.dram_tensor("cnt", [128, qw], mybir.dt.uint8,
                          kind="ExternalOutput").ap()

    with tile.TileContext(nc) as tc:
        with tc.tile_pool(name="stream", bufs=2) as stream, \
             tc.tile_pool(name="sqp", bufs=2) as sqp, \
             tc.tile_pool(name="accp", bufs=2) as accp, \
             tc.tile_pool(name="small", bufs=1) as small, \
             tc.tile_pool(name="ps", bufs=2, space="PSUM") as pstat:
            ident = small.tile([128, 128], mybir.dt.float32)
            nc.sync.dma_start(out=ident, in_=identd)
            # broadcast thresholds to every partition: [128, 2*5*4]
            thb = small.tile([128, 2 * 5 * NCH], mybir.dt.float32)
            nc.sync.dma_start(
                out=thb,
                in_=bass.AP(tensor=th.tensor, offset=0, ap=[[0, 128], [1, 2 * 5 * NCH]]),
            )
            for it in range(niters):
                ld = stream.tile([128, NROWS, free], mybir.dt.float32, tag="ld")
                src = bass.AP(
                    tensor=shard.tensor,
                    offset=it * chunk * NCH,
                    ap=[[padw_pp * NCH, 128], [128 * padw_pp * NCH, NROWS],
                        [NCH, chunk], [1, NCH]],
                )
                nc.sync.dma_start(out=ld.rearrange("p t (c k) -> p t c k", k=NCH), in_=src)

                # ---- q over all 10 rows ----
                sq = sqp.tile([128, NROWS, free], mybir.dt.float32, tag="sq")
                nc.scalar.activation(out=sq, in_=ld,
                                     func=mybir.ActivationFunctionType.Square)
                sum_acc = pstat.tile([128, free], mybir.dt.float32, tag="sum",
                                     name="sum_ps")
                ssq_acc = pstat.tile([128, free], mybir.dt.float32, tag="ssq",
                                     name="ssq_ps")
                for t in range(NROWS):
                    nc.tensor.matmul(sum_acc, lhsT=ident, rhs=ld[:, t, :],
                                     start=(t == 0), stop=(t == NROWS - 1))
                for t in range(NROWS):
                    nc.tensor.matmul(ssq_acc, lhsT=ident, rhs=sq[:, t, :],
                                     start=(t == 0), stop=(t == NROWS - 1))
                t1 = accp.tile([128, free], mybir.dt.float32, tag="t1")
                # sum^2 via ACT Square: single PSUM read, exact x*x
                nc.scalar.activation(out=t1, in_=sum_acc,
                                     func=mybir.ActivationFunctionType.Square)
                nc.vector.scalar_tensor_tensor(
                    out=t1, in0=t1, scalar=-0.1, in1=ssq_acc,
                    op0=mybir.AluOpType.mult, op1=mybir.AluOpType.add,
                )
                nc.sync.dma_start(out=qvar[:, it * free:(it + 1) * free], in_=t1)

                # ---- bracket counts over rows 0-4 ----
                accs = []
                for k in range(2):  # 0 = lo, 1 = hi
                    acc = accp.tile([128, free], mybir.dt.float32, tag=f"acc{k}")
                    cmp = accp.tile([128, free], mybir.dt.float32, tag=f"cmp{k}")
                    for t in range(5):
                        thv = bass.AP(tensor=thb.tensor,
                                      offset=thb.offset + (k * 5 + t) * NCH,
                                      ap=[thb.ap[0], [0, chunk], [1, NCH]])
                        dst = acc if t == 0 else cmp
                        nc.vector.scalar_tensor_tensor(
                            out=dst.rearrange("p (c k) -> p c k", k=NCH),
                            in0=thv, scalar=0.0,
                            in1=ld[:, t, :].rearrange("p (c k) -> p c k", k=NCH),
                            op0=mybir.AluOpType.add, op1=mybir.AluOpType.is_le,
                        )
                        if t > 0:
                            nc.vector.tensor_tensor(out=acc, in0=acc, in1=cmp,
                                                    op=mybir.AluOpType.add)
                    accs.append(acc)
                pk = accp.tile([128, free], mybir.dt.uint8, tag="pk")
                # pk = cnt_lo + 8*cnt_hi (integers <= 45, exact in uint8)
                nc.vector.scalar_tensor_tensor(
                    out=pk, in0=accs[1], scalar=8.0, in1=accs[0],
                    op0=mybir.AluOpType.mult, op1=mybir.AluOpType.add,
                )
                nc.sync.dma_start(out=cnto[:, it * free:(it + 1) * free], in_=pk)
    if split_waits:
        _split_sync_waits(nc)
    return nc


def _trim(arr128, width, padw_pp=PADW_PP):
    """[128, padw_pp*4] core output -> (width, 4)."""
    return arr128.reshape(128 * padw_pp, NCH)[:width]


def _logsumexp_f32(v):
    m = np.max(v)
    return np.float32(np.log(np.sum(np.exp(v - m, dtype=np.float32), dtype=np.float32)) + m)


def _numpy_fallback(logits, x, delta):
    logits = np.asarray(logits, dtype=np.float32)
    x = np.asarray(x, dtype=np.float32)
    delta = np.float32(delta)
    n = logits.shape[1]
    med = np.sort(logits, axis=1)[:, (n - 1) // 2, :]
    std = np.asarray(logits, dtype=np.float32).std(axis=0, ddof=1).astype(np.float32)
    std_med = np.sort(std, axis=0)[(n - 1) // 2, :]
    thresh = med[:, None, :]
    above = (logits >= thresh + FACTOR * std_med) & (logits >= thresh + delta / 2)
    cls = above.astype(np.int32)
    s = cls[:5].sum(axis=0)
    mode = (s >= 3).astype(np.float32)
    c = np.broadcast_to(mode[None], logits.shape).astype(np.float32)
    xs = np.concatenate([np.zeros((x.shape[0], 1), x.dtype), x], axis=1)
    dx = delta * c + xs[:, None, :]
    outs = []
    for i in range(4):
        oth = [j for j in range(4) if j != i]
        m = dx[..., oth].max(axis=-1)
        lse = np.log(np.sum(np.exp(dx[..., oth] - m[..., None]), axis=-1)) + m
        outs.append(dx[..., i] - lse)
    return np.stack(outs, axis=-1).astype(np.float32), c


def _exact_medians(logits, out):
    """Exact lower medians med[t,ch] for t<5 via introselect (bit-exact vs
    the reference's sort-based torch_median). Runs on a worker thread while
    the device launch is in flight."""
    k = (N - 1) // 2
    for t in range(5):
        p = np.partition(logits[t], k, axis=0)
        out[t] = p[k]


def kernel(logits, x, delta):
    logits = np.ascontiguousarray(np.asarray(logits, dtype=np.float32))
    x = np.asarray(x, dtype=np.float32)
    dval = float(np.asarray(delta))
    if dval != 0.0 or logits.shape != (10, N, 4):
        return _numpy_fallback(logits, x, delta)

    from concourse.bass_utils import run_bass_kernel_spmd

    def _run(nc, in_maps, cores):
        # a wedged accelerator session recovers on a fresh NRT attempt
        import time as _t
        try:
            return run_bass_kernel_spmd(nc, in_maps, core_ids=cores)
        except Exception:
            _t.sleep(5)
            return run_bass_kernel_spmd(nc, in_maps, core_ids=cores)

    import time as _time
    cores = list(range(NCORES))

    # ---------- build the device programs on a worker (pure-python) while
    # the main thread stages inputs (numpy memcpy, releases the GIL). Two
    # independent nc objects, built sequentially (deterministic), so the two
    # overlapped launches share no mutable builder state. ----------
    built = {}

    def _builder():
        built["nc_a"] = build_fused()
        built["nc_b"] = build_fused()

    bt = threading.Thread(target=_builder)
    bt.start()

    # ---------- host: estimated threshold bracket from a 1/16 subsample ----
    sub = logits[:, ::16, :]
    med_est = np.median(sub, axis=1).astype(np.float32)          # (10, 4)
    q_sub = (sub.var(axis=0, ddof=1) * np.float32(9)).astype(np.float32)
    qmed_est = np.median(q_sub, axis=0).astype(np.float32)
    std_med_est = np.sqrt(qmed_est / np.float32(9)).astype(np.float32)
    t_est = med_est[:5] + FACTOR * std_med_est[None, :]          # (5, 4)
    th = np.stack([t_est - DELTA_BRK, t_est + DELTA_BRK]).astype(np.float32)

    # ---------- stage padded shards (two overlapped half-launches) ----------
    ident = np.eye(128, dtype=np.float32)
    in_h = [[], []]
    for h in range(2):
        for c in cores:
            sh = np.zeros((NROWS, SHARD_PAD, NCH), dtype=np.float32)
            lo = c * SHARD + h * SHARD_H
            sh[:, :SHARD_H, :] = logits[:, lo:lo + SHARD_H, :]
            in_h[h].append({"shardpad": sh, "th": th, "ident": ident})
    bt.join()
    ncs = [built["nc_a"], built["nc_b"]]

    # ---------- two overlapped launches (transfer of one hides exec+fetch
    # of the other); exact meds overlap them both ----------
    med = np.empty((5, NCH), dtype=np.float32)
    mt = threading.Thread(target=_exact_medians, args=(logits, med))
    mt.start()
    res = [None, None]

    def _half(h):
        res[h] = _run(ncs[h], in_h[h], cores)

    _t = _time.time()
    t0 = threading.Thread(target=_half, args=(0,))
    t0.start()
    _half(1)
    t0.join()
    LAST_RUN_TIMES.append(_time.time() - _t)
    mt.join()

    qvar = np.concatenate(
        [_trim(res[h].results[c]["qvar"], SHARD_H)
         for c in cores for h in range(2)], axis=0
    )  # (N, 4) float32
    pk = np.concatenate(
        [_trim(res[h].results[c]["cnt"], SHARD_H)
         for c in cores for h in range(2)],
        axis=0,
    ).astype(np.int32)  # (N, 4) packed cnt_lo + 8*cnt_hi
    cnt_lo = pk & 7
    cnt_hi = pk >> 3
    if np.any(cnt_lo > 5) or np.any(cnt_hi > cnt_lo):
        # malformed device counts (never): exact host re-derivation
        return _numpy_fallback(logits, x, delta)

    # ---------- host: exact qmed -> exact thresholds -> mode ----------
    k = (N - 1) // 2
    qmed = np.empty(NCH, dtype=np.float32)
    for ch in range(NCH):
        qmed[ch] = np.partition(np.ascontiguousarray(qvar[:, ch]), k)[k]
    std_med = np.sqrt(qmed / np.float32(9)).astype(np.float32)
    t_exact = med + FACTOR * std_med[None, :]          # (5, 4) f32, ref formula

    if not (np.all(th[0] <= t_exact) and np.all(t_exact <= th[1])
            and np.all(std_med > 0)):
        # bracket miss (never for N(0,1) inputs): exact host re-derivation
        return _numpy_fallback(logits, x, delta)

    mode = (cnt_hi >= 3)
    uncertain = (cnt_lo >= 3) & ~mode                  # bracket straddles >=3
    un_n, un_ch = np.nonzero(uncertain)
    if un_n.size:
        vals = logits[:5, un_n, un_ch]                 # (5, K)
        s = (vals >= t_exact[:, un_ch]).sum(axis=0)
        mode[un_n, un_ch] = s >= 3
    mode = mode.astype(np.float32)

    # ---------- host assembly ----------
    xs = np.concatenate([np.zeros((x.shape[0], 1), np.float32), x], axis=1)
    table = np.zeros((10, 4), dtype=np.float32)
    for t in range(10):
        for i in range(4):
            oth = [j for j in range(4) if j != i]
            table[t, i] = xs[t, i] - _logsumexp_f32(xs[t, oth])
    out_full = np.broadcast_to(table[:, None, :], (10, N, 4))
    c_full = np.broadcast_to(mode[None], (10, N, 4))
    return out_full, c_full
